# revision 7
# baseline (speedup 1.0000x reference)
"""Trainium2 Bass kernel for i1e(z) (exponentially scaled modified Bessel I1).

Input: z float32 (32, 1024, 1024), values in [0.1, 10.1] (positive).
Output: i1e(z), same shape/dtype, matching the A&S-style reference to
~1.4e-2 pointwise / ~6.4e-3 norm relative error (harness gate is 2e-2).

Strategy (per core, trivially data-parallel over the leading batch axis):
  - Each of 8 cores gets 4 batches = 4Mi elements, viewed as [128, 32768] f32.
  - Single-branch approximation in the log domain:
        i1e(x) ~= exp(P4(ln x)),   P4 a quartic fit on [ln 0.1, ln 10.1]
    refit against the exact bf16-quantized evaluation chain below.  The log
    transform linearizes both asymptotics (i1e ~ x/2 near 0, ~0.4/sqrt(x)
    at inf), which is what makes a mere quartic sufficient.
  - Per tile (ScalarE ops from the natural_log_exp_and_others table set):
        u = Ln(x)                 ACT, f32 -> bf16
        q = (ALPHA*u + BETA)^2    completed-square quartic head, two ways:
              D-tiles: t = TS(u*ALPHA + BETA); q = TT(t*t)     DVE bf16 4x/2x
              B-tiles: q = Square(ALPHA*u + BETA)              ACT
        v = (q + C)*u             DVE STT bf16 (2x mode)
        w = (v + D)*u             DVE STT, f32 out (keeps Exp input + bias
                                  const in f32; also trims bf16 noise)
        out = Exp(w + BE)         ACT, f32
    so P4 = (ALPHA*u+BETA)^2*u^2 + C*u^2 + D*u + BE spans general quartics.
  - 3 of 8 tiles take the B (ACT-square) path: measured engine rates
    (ACT ~(N+352)/1.2GHz dtype-independent; DVE bf16 STT/TT 2x, TS 4x;
    f32 STT 1x) balance at ACT ~70us, DVE ~70us per core-pass, both under
    the ~85-95us/core HBM roofline (16 MiB in + 16 MiB out), leaving the
    kernel cleanly DMA-bound.
"""

import numpy as np

import concourse.bass as bass
import concourse.tile as tile
from concourse import mybir
from concourse.bass_utils import run_bass_kernel_spmd

AF = mybir.ActivationFunctionType
ALU = mybir.AluOpType
F32 = mybir.dt.float32
BF16 = mybir.dt.bfloat16

N_CORES = 8
P = 128              # SBUF partitions
FD_TOTAL = 32768     # free-dim elements per partition per core (4Mi total)
TILE_FD = 4096       # free-dim per tile
X_BUFS = 4           # input-tile ring depth (DMA prefetch runway)
OUT_BUFS = 3         # output-tile ring depth
TMP_BUFS = 2
N_ACT_SQ = 3         # of every 8 tiles, this many use the ACT-square path
STORE_SCALAR = True  # issue stores from the ACT HWDGE ring: the trigger sits
                     # right after Exp on the same engine so its wait is
                     # pre-satisfied, and stores never head-of-line-block
                     # input loads queued on the SP HWDGE ring

# Quartic P4(u) ~= ln(i1e(e^u)) on u in [ln 0.1, ln 10.1], minimax-refit
# through the exact bf16 evaluation chain (see module docstring).
ALPHA = 0.10338272154331207
BETA = -0.012421127408742905
C = -0.2503415644168854
D = 0.2245168834924698
BE = -1.5742369890213013

ACT_BIAS_CONSTS = [BETA, BE]

_CACHED_NC = None


def build_nc(reps: int = 1):
    nc = bass.Bass(trn_type="TRN2")
    x_ext = nc.declare_dram_parameter("x", [P, FD_TOTAL], F32, isOutput=False)
    o_ext = nc.declare_dram_parameter("o", [P, FD_TOTAL], F32, isOutput=True)

    # Register activation-bias constants as const APs, mirroring
    # Bass.__init__'s register_const_ap for 0.0/1.0.
    for i, v in enumerate(ACT_BIAS_CONSTS):
        tns = nc.alloc_sbuf_tensor(f"const-f32-bias{i}", [P, 1], F32)
        nc.gpsimd.memset(tns.ap(), v)
        nc.const_aps.aps[(F32, v)] = tns.ap()
    nc.all_engine_barrier()

    n_tiles = FD_TOTAL // TILE_FD
    store_engine = nc.scalar if STORE_SCALAR else nc.sync
    with tile.TileContext(nc) as tc:
        with (
            tc.tile_pool(name="iox", bufs=X_BUFS) as iox,
            tc.tile_pool(name="ioo", bufs=OUT_BUFS) as ioo,
            tc.tile_pool(name="tmp", bufs=TMP_BUFS) as tmp,
        ):
            for i in range(n_tiles * reps):
                i = i % n_tiles
                sl = bass.ts(i, TILE_FD)

                x = iox.tile([P, TILE_FD], F32, tag="x")
                nc.sync.dma_start(x[:], x_ext[:, sl])

                u = tmp.tile([P, TILE_FD], BF16, tag="u")
                nc.scalar.activation(u[:], x[:], AF.Ln)

                q = tmp.tile([P, TILE_FD], BF16, tag="q")
                if i % 8 >= 8 - N_ACT_SQ:
                    nc.scalar.activation(q[:], u[:], AF.Square,
                                         scale=ALPHA, bias=BETA)
                else:
                    t = tmp.tile([P, TILE_FD], BF16, tag="t")
                    nc.vector.tensor_scalar(t[:], u[:], ALPHA, BETA,
                                            ALU.mult, ALU.add)
                    nc.vector.tensor_tensor(q[:], t[:], t[:], ALU.mult)

                nc.vector.scalar_tensor_tensor(
                    q[:], q[:], C, u[:], ALU.add, ALU.mult)
                w = tmp.tile([P, TILE_FD], F32, tag="w")
                nc.vector.scalar_tensor_tensor(
                    w[:], q[:], D, u[:], ALU.add, ALU.mult)

                out = ioo.tile([P, TILE_FD], F32, tag="out")
                nc.scalar.activation(out[:], w[:], AF.Exp, bias=BE)

                store_engine.dma_start(o_ext[:, sl], out[:])

    _split_multi_waits(nc)
    return nc


# TPB compute-instruction ISA formats carry at most ONE sync-wait, but Tile's
# semaphore assignment can attach several (its wait minimality is per-proc,
# not transitive).  Hoist all but one wait onto an InstNoOp inserted right
# before the offending instruction on the same engine.
def _split_multi_waits(nc):
    for bb in nc.main_func.blocks:
        insts = bb.instructions
        i = 0
        while i < len(insts):
            inst = insts[i]
            si = inst.sync_info
            if si is not None and len(si.on_wait) > 1:
                for w in si.on_wait[:-1]:
                    nop = mybir.InstNoOp(
                        name=nc.get_next_instruction_name(),
                        text_hint="wait_split",
                        bass_nofuse=True,
                        engine=inst.engine,
                        sync_info=mybir.SyncInfo(on_wait=[w], on_update=[]),
                    )
                    insts.insert(i, nop)
                    i += 1
                si.on_wait = [si.on_wait[-1]]
            i += 1


def kernel(z: np.ndarray) -> np.ndarray:
    global _CACHED_NC
    assert z.shape == (32, 1024, 1024) and z.dtype == np.float32
    if _CACHED_NC is None:
        _CACHED_NC = build_nc()
    nc = _CACHED_NC

    per_core = 32 // N_CORES
    shards = z.reshape(N_CORES, per_core * 1024 * 1024).reshape(N_CORES, P, FD_TOTAL)
    in_maps = [{"x": np.ascontiguousarray(shards[k])} for k in range(N_CORES)]
    res = run_bass_kernel_spmd(nc, in_maps, list(range(N_CORES))).results
    out = np.concatenate(
        [res[k]["o"].reshape(per_core, 1024, 1024) for k in range(N_CORES)], axis=0
    )
    return out.astype(np.float32)


# revision 10
# speedup vs baseline: 1.1260x; 1.1260x over previous
"""Trainium2 Bass kernel for i1e(z) (exponentially scaled modified Bessel I1).

Input: z float32 (32, 1024, 1024), values in [0.1, 10.1] (positive).
Output: i1e(z), same shape/dtype, matching the A&S-style reference to
~1.4e-2 pointwise / ~6.4e-3 norm relative error (harness gate is 2e-2).

Strategy (per core, trivially data-parallel over the leading batch axis):
  - Each of 8 cores gets 4 batches = 4Mi elements, viewed as [128, 32768] f32.
  - Single-branch approximation in the log domain:
        i1e(x) ~= exp(P4(ln x)),   P4 a quartic fit on [ln 0.1, ln 10.1]
    refit against the exact bf16-quantized evaluation chain below.  The log
    transform linearizes both asymptotics (i1e ~ x/2 near 0, ~0.4/sqrt(x)
    at inf), which is what makes a mere quartic sufficient.
  - Per tile (ScalarE ops from the natural_log_exp_and_others table set):
        u = Ln(x)                 ACT, f32 -> bf16
        q = (ALPHA*u + BETA)^2    completed-square quartic head, two ways:
              D-tiles: t = TS(u*ALPHA + BETA); q = TT(t*t)     DVE bf16 4x/2x
              B-tiles: q = Square(ALPHA*u + BETA)              ACT
        v = (q + C)*u             DVE STT bf16 (2x mode)
        w = (v + D)*u             DVE STT, f32 out (keeps Exp input + bias
                                  const in f32; also trims bf16 noise)
        out = Exp(w + BE)         ACT, f32
    so P4 = (ALPHA*u+BETA)^2*u^2 + C*u^2 + D*u + BE spans general quartics.
  - 3 of 8 tiles take the B (ACT-square) path: measured engine rates
    (ACT ~(N+352)/1.2GHz dtype-independent; DVE bf16 STT/TT 2x, TS 4x;
    f32 STT 1x) balance at ACT ~70us, DVE ~70us per core-pass, both under
    the ~85-95us/core HBM roofline (16 MiB in + 16 MiB out), leaving the
    kernel cleanly DMA-bound.
"""

import numpy as np

import concourse.bass as bass
import concourse.tile as tile
from concourse import mybir
from concourse.bass_utils import run_bass_kernel_spmd

AF = mybir.ActivationFunctionType
ALU = mybir.AluOpType
F32 = mybir.dt.float32
BF16 = mybir.dt.bfloat16

N_CORES = 8
P = 128              # SBUF partitions
FD_TOTAL = 32768     # free-dim elements per partition per core (4Mi total)
TILE_FD = 4096       # free-dim per tile
X_BUFS = 4           # input-tile ring depth (DMA prefetch runway)
OUT_BUFS = 3         # output-tile ring depth
TMP_BUFS = 2
N_ACT_SQ = 0         # of every 8 tiles, this many use the ACT-square path
STORE_SCALAR = True  # issue stores from the ACT HWDGE ring: the trigger sits
                     # right after Exp on the same engine so its wait is
                     # pre-satisfied, and stores never head-of-line-block
                     # input loads queued on the SP HWDGE ring
CAST_LOAD = True     # SWDGE (gpsimd-ring) input DMA with f32->bf16 cast:
                     # HBM still reads f32, but SBUF-side write bytes halve,
                     # relieving the 435 GB/s SBUF AXI fabric ceiling; loads
                     # get their own issue ring (gpsimd) with no dependencies,
                     # so three DMA paths (gpsimd loads / ACT stores / SP
                     # idle) can never head-of-line-block each other.

# Quartic P4(u) ~= ln(i1e(e^u)) on u in [ln 0.1, ln 10.1], minimax-refit
# through the exact bf16 evaluation chain (incl. the cast-on-load input
# quantization; see module docstring).
if CAST_LOAD:
    ALPHA = 0.1032966673374176
    BETA = -0.012588093057274818
    C = -0.2503528296947479
    D = 0.22434590756893158
    BE = -1.5741204023361206
else:
    ALPHA = 0.10338272154331207
    BETA = -0.012421127408742905
    C = -0.2503415644168854
    D = 0.2245168834924698
    BE = -1.5742369890213013

ACT_BIAS_CONSTS = [BETA, BE]

_CACHED_NC = None


def build_nc(reps: int = 1):
    nc = bass.Bass(trn_type="TRN2")
    x_ext = nc.declare_dram_parameter("x", [P, FD_TOTAL], F32, isOutput=False)
    o_ext = nc.declare_dram_parameter("o", [P, FD_TOTAL], F32, isOutput=True)

    # Register activation-bias constants as const APs, mirroring
    # Bass.__init__'s register_const_ap for 0.0/1.0.
    for i, v in enumerate(ACT_BIAS_CONSTS):
        tns = nc.alloc_sbuf_tensor(f"const-f32-bias{i}", [P, 1], F32)
        nc.gpsimd.memset(tns.ap(), v)
        nc.const_aps.aps[(F32, v)] = tns.ap()
    nc.all_engine_barrier()

    n_tiles = FD_TOTAL // TILE_FD
    store_engine = nc.scalar if STORE_SCALAR else nc.sync
    with tile.TileContext(nc) as tc:
        with (
            tc.tile_pool(name="iox", bufs=X_BUFS) as iox,
            tc.tile_pool(name="ioo", bufs=OUT_BUFS) as ioo,
            tc.tile_pool(name="tmp", bufs=TMP_BUFS) as tmp,
        ):
            for i in range(n_tiles * reps):
                i = i % n_tiles
                sl = bass.ts(i, TILE_FD)

                if CAST_LOAD:
                    x = iox.tile([P, TILE_FD], BF16, tag="x")
                    nc.gpsimd.dma_start(x[:], x_ext[:, sl])
                else:
                    x = iox.tile([P, TILE_FD], F32, tag="x")
                    nc.sync.dma_start(x[:], x_ext[:, sl])

                u = tmp.tile([P, TILE_FD], BF16, tag="u")
                nc.scalar.activation(u[:], x[:], AF.Ln)

                q = tmp.tile([P, TILE_FD], BF16, tag="q")
                if i % 8 >= 8 - N_ACT_SQ:
                    nc.scalar.activation(q[:], u[:], AF.Square,
                                         scale=ALPHA, bias=BETA)
                else:
                    t = tmp.tile([P, TILE_FD], BF16, tag="t")
                    nc.vector.tensor_scalar(t[:], u[:], ALPHA, BETA,
                                            ALU.mult, ALU.add)
                    nc.vector.tensor_tensor(q[:], t[:], t[:], ALU.mult)

                nc.vector.scalar_tensor_tensor(
                    q[:], q[:], C, u[:], ALU.add, ALU.mult)
                w = tmp.tile([P, TILE_FD], BF16 if CAST_LOAD else F32, tag="w")
                nc.vector.scalar_tensor_tensor(
                    w[:], q[:], D, u[:], ALU.add, ALU.mult)

                out = ioo.tile([P, TILE_FD], F32, tag="out")
                nc.scalar.activation(out[:], w[:], AF.Exp, bias=BE)

                store_engine.dma_start(o_ext[:, sl], out[:])

    _split_multi_waits(nc)
    return nc


# TPB compute-instruction ISA formats carry at most ONE sync-wait, but Tile's
# semaphore assignment can attach several (its wait minimality is per-proc,
# not transitive).  Hoist all but one wait onto an InstNoOp inserted right
# before the offending instruction on the same engine.
def _split_multi_waits(nc):
    for bb in nc.main_func.blocks:
        insts = bb.instructions
        i = 0
        while i < len(insts):
            inst = insts[i]
            si = inst.sync_info
            if si is not None and len(si.on_wait) > 1:
                for w in si.on_wait[:-1]:
                    nop = mybir.InstNoOp(
                        name=nc.get_next_instruction_name(),
                        text_hint="wait_split",
                        bass_nofuse=True,
                        engine=inst.engine,
                        sync_info=mybir.SyncInfo(on_wait=[w], on_update=[]),
                    )
                    insts.insert(i, nop)
                    i += 1
                si.on_wait = [si.on_wait[-1]]
            i += 1


def kernel(z: np.ndarray) -> np.ndarray:
    global _CACHED_NC
    assert z.shape == (32, 1024, 1024) and z.dtype == np.float32
    if _CACHED_NC is None:
        _CACHED_NC = build_nc()
    nc = _CACHED_NC

    per_core = 32 // N_CORES
    shards = z.reshape(N_CORES, per_core * 1024 * 1024).reshape(N_CORES, P, FD_TOTAL)
    in_maps = [{"x": np.ascontiguousarray(shards[k])} for k in range(N_CORES)]
    res = run_bass_kernel_spmd(nc, in_maps, list(range(N_CORES))).results
    out = np.concatenate(
        [res[k]["o"].reshape(per_core, 1024, 1024) for k in range(N_CORES)], axis=0
    )
    return out.astype(np.float32)


# revision 12
# speedup vs baseline: 1.1531x; 1.0241x over previous
"""Trainium2 Bass kernel for i1e(z) (exponentially scaled modified Bessel I1).

Input: z float32 (32, 1024, 1024), values in [0.1, 10.1] (positive).
Output: i1e(z), same shape/dtype, matching the A&S-style reference to
~1.4e-2 pointwise / ~6.4e-3 norm relative error (harness gate is 2e-2).

Strategy (per core, trivially data-parallel over the leading batch axis):
  - Each of 8 cores gets 4 batches = 4Mi elements, viewed as [128, 32768] f32.
  - Single-branch approximation in the log domain:
        i1e(x) ~= exp(P4(ln x)),   P4 a quartic fit on [ln 0.1, ln 10.1]
    refit against the exact bf16-quantized evaluation chain below.  The log
    transform linearizes both asymptotics (i1e ~ x/2 near 0, ~0.4/sqrt(x)
    at inf), which is what makes a mere quartic sufficient.
  - Per tile (ScalarE ops from the natural_log_exp_and_others table set):
        u = Ln(x)                 ACT, f32 -> bf16
        q = (ALPHA*u + BETA)^2    completed-square quartic head, two ways:
              D-tiles: t = TS(u*ALPHA + BETA); q = TT(t*t)     DVE bf16 4x/2x
              B-tiles: q = Square(ALPHA*u + BETA)              ACT
        v = (q + C)*u             DVE STT bf16 (2x mode)
        w = (v + D)*u             DVE STT, f32 out (keeps Exp input + bias
                                  const in f32; also trims bf16 noise)
        out = Exp(w + BE)         ACT, f32
    so P4 = (ALPHA*u+BETA)^2*u^2 + C*u^2 + D*u + BE spans general quartics.
  - 3 of 8 tiles take the B (ACT-square) path: measured engine rates
    (ACT ~(N+352)/1.2GHz dtype-independent; DVE bf16 STT/TT 2x, TS 4x;
    f32 STT 1x) balance at ACT ~70us, DVE ~70us per core-pass, both under
    the ~85-95us/core HBM roofline (16 MiB in + 16 MiB out), leaving the
    kernel cleanly DMA-bound.
"""

import numpy as np

import concourse.bass as bass
import concourse.tile as tile
from concourse import mybir
from concourse.bass_utils import run_bass_kernel_spmd

AF = mybir.ActivationFunctionType
ALU = mybir.AluOpType
F32 = mybir.dt.float32
BF16 = mybir.dt.bfloat16

N_CORES = 8
P = 128              # SBUF partitions
FD_TOTAL = 32768     # free-dim elements per partition per core (4Mi total)
TILE_FD = 4096       # free-dim per tile
X_BUFS = 4           # input-tile ring depth (DMA prefetch runway)
OUT_BUFS = 3         # output-tile ring depth
TMP_BUFS = 2
N_ACT_SQ = 3         # of every 8 tiles, this many use the ACT-square path
                     # (balances ACT ~70us vs DVE ~70us per core-pass)
STORE_SCALAR = True  # issue stores from the ACT HWDGE ring: the trigger sits
                     # right after Exp on the same engine so its wait is
                     # pre-satisfied, and stores never head-of-line-block
                     # input loads queued on the SP HWDGE ring
CAST_LOAD = False    # SWDGE (gpsimd-ring) input DMA with f32->bf16 cast.
                     # Measured NOT faster: the DMA bound tracks the f32
                     # bytes through the SDMA/HBM path regardless of the
                     # SBUF-side dtype, and the SWDGE cast path adds ~2-6us
                     # per pass, so plain HWDGE f32 loads win.

# Quartic P4(u) ~= ln(i1e(e^u)) on u in [ln 0.1, ln 10.1], minimax-refit
# through the exact bf16 evaluation chain (incl. the cast-on-load input
# quantization; see module docstring).
if CAST_LOAD:
    ALPHA = 0.1032966673374176
    BETA = -0.012588093057274818
    C = -0.2503528296947479
    D = 0.22434590756893158
    BE = -1.5741204023361206
else:
    ALPHA = 0.10338272154331207
    BETA = -0.012421127408742905
    C = -0.2503415644168854
    D = 0.2245168834924698
    BE = -1.5742369890213013

ACT_BIAS_CONSTS = [BETA, BE]

_CACHED_NC = None


def build_nc(reps: int = 1):
    nc = bass.Bass(trn_type="TRN2")
    x_ext = nc.declare_dram_parameter("x", [P, FD_TOTAL], F32, isOutput=False)
    o_ext = nc.declare_dram_parameter("o", [P, FD_TOTAL], F32, isOutput=True)

    # Register activation-bias constants as const APs, mirroring
    # Bass.__init__'s register_const_ap for 0.0/1.0.
    for i, v in enumerate(ACT_BIAS_CONSTS):
        tns = nc.alloc_sbuf_tensor(f"const-f32-bias{i}", [P, 1], F32)
        nc.gpsimd.memset(tns.ap(), v)
        nc.const_aps.aps[(F32, v)] = tns.ap()
    nc.all_engine_barrier()

    n_tiles = FD_TOTAL // TILE_FD
    store_engine = nc.scalar if STORE_SCALAR else nc.sync
    with tile.TileContext(nc) as tc:
        with (
            tc.tile_pool(name="iox", bufs=X_BUFS) as iox,
            tc.tile_pool(name="ioo", bufs=OUT_BUFS) as ioo,
            tc.tile_pool(name="tmp", bufs=TMP_BUFS) as tmp,
        ):
            for i in range(n_tiles * reps):
                i = i % n_tiles
                sl = bass.ts(i, TILE_FD)

                if CAST_LOAD:
                    x = iox.tile([P, TILE_FD], BF16, tag="x")
                    nc.gpsimd.dma_start(x[:], x_ext[:, sl])
                else:
                    x = iox.tile([P, TILE_FD], F32, tag="x")
                    nc.sync.dma_start(x[:], x_ext[:, sl])

                u = tmp.tile([P, TILE_FD], BF16, tag="u")
                nc.scalar.activation(u[:], x[:], AF.Ln)

                q = tmp.tile([P, TILE_FD], BF16, tag="q")
                if i % 8 >= 8 - N_ACT_SQ:
                    nc.scalar.activation(q[:], u[:], AF.Square,
                                         scale=ALPHA, bias=BETA)
                else:
                    t = tmp.tile([P, TILE_FD], BF16, tag="t")
                    nc.vector.tensor_scalar(t[:], u[:], ALPHA, BETA,
                                            ALU.mult, ALU.add)
                    nc.vector.tensor_tensor(q[:], t[:], t[:], ALU.mult)

                nc.vector.scalar_tensor_tensor(
                    q[:], q[:], C, u[:], ALU.add, ALU.mult)
                w = tmp.tile([P, TILE_FD], BF16 if CAST_LOAD else F32, tag="w")
                nc.vector.scalar_tensor_tensor(
                    w[:], q[:], D, u[:], ALU.add, ALU.mult)

                out = ioo.tile([P, TILE_FD], F32, tag="out")
                nc.scalar.activation(out[:], w[:], AF.Exp, bias=BE)

                store_engine.dma_start(o_ext[:, sl], out[:])

    _split_multi_waits(nc)
    return nc


# TPB compute-instruction ISA formats carry at most ONE sync-wait, but Tile's
# semaphore assignment can attach several (its wait minimality is per-proc,
# not transitive).  Hoist all but one wait onto an InstNoOp inserted right
# before the offending instruction on the same engine.
def _split_multi_waits(nc):
    for bb in nc.main_func.blocks:
        insts = bb.instructions
        i = 0
        while i < len(insts):
            inst = insts[i]
            si = inst.sync_info
            if si is not None and len(si.on_wait) > 1:
                for w in si.on_wait[:-1]:
                    nop = mybir.InstNoOp(
                        name=nc.get_next_instruction_name(),
                        text_hint="wait_split",
                        bass_nofuse=True,
                        engine=inst.engine,
                        sync_info=mybir.SyncInfo(on_wait=[w], on_update=[]),
                    )
                    insts.insert(i, nop)
                    i += 1
                si.on_wait = [si.on_wait[-1]]
            i += 1


def kernel(z: np.ndarray) -> np.ndarray:
    global _CACHED_NC
    assert z.shape == (32, 1024, 1024) and z.dtype == np.float32
    if _CACHED_NC is None:
        _CACHED_NC = build_nc()
    nc = _CACHED_NC

    per_core = 32 // N_CORES
    shards = z.reshape(N_CORES, per_core * 1024 * 1024).reshape(N_CORES, P, FD_TOTAL)
    in_maps = [{"x": np.ascontiguousarray(shards[k])} for k in range(N_CORES)]
    res = run_bass_kernel_spmd(nc, in_maps, list(range(N_CORES))).results
    out = np.concatenate(
        [res[k]["o"].reshape(per_core, 1024, 1024) for k in range(N_CORES)], axis=0
    )
    return out.astype(np.float32)


# revision 14
# speedup vs baseline: 1.7285x; 1.4989x over previous
"""Trainium2 Bass kernel for i1e(z) (exponentially scaled modified Bessel I1).

Input: z float32 (32, 1024, 1024), values in [0.1, 10.1] (positive).
Output: i1e(z), same shape/dtype, matching the A&S-style reference to
~1.4e-2 pointwise / ~6.4e-3 norm relative error (harness gate is 2e-2).

Strategy (per core, trivially data-parallel over the leading batch axis):
  - Each of 8 cores gets 4 batches = 4Mi elements, viewed as [128, 32768] f32.
  - Single-branch approximation in the log domain:
        i1e(x) ~= exp(P4(ln x)),   P4 a quartic fit on [ln 0.1, ln 10.1]
    refit against the exact bf16-quantized evaluation chain below.  The log
    transform linearizes both asymptotics (i1e ~ x/2 near 0, ~0.4/sqrt(x)
    at inf), which is what makes a mere quartic sufficient.
  - Per tile (ScalarE ops from the natural_log_exp_and_others table set):
        u = Ln(x)                 ACT, f32 -> bf16
        q = (ALPHA*u + BETA)^2    completed-square quartic head, two ways:
              D-tiles: t = TS(u*ALPHA + BETA); q = TT(t*t)     DVE bf16 4x/2x
              B-tiles: q = Square(ALPHA*u + BETA)              ACT
        v = (q + C)*u             DVE STT bf16 (2x mode)
        w = (v + D)*u             DVE STT, f32 out (keeps Exp input + bias
                                  const in f32; also trims bf16 noise)
        out = Exp(w + BE)         ACT, f32
    so P4 = (ALPHA*u+BETA)^2*u^2 + C*u^2 + D*u + BE spans general quartics.
  - 3 of 8 tiles take the B (ACT-square) path: measured engine rates
    (ACT ~(N+352)/1.2GHz dtype-independent; DVE bf16 STT/TT 2x, TS 4x;
    f32 STT 1x) balance at ACT ~70us, DVE ~70us per core-pass, both under
    the ~85-95us/core HBM roofline (16 MiB in + 16 MiB out), leaving the
    kernel cleanly DMA-bound.
"""

import numpy as np

import concourse.bass as bass
import concourse.tile as tile
from concourse import mybir
from concourse.bass_utils import run_bass_kernel_spmd

AF = mybir.ActivationFunctionType
ALU = mybir.AluOpType
F32 = mybir.dt.float32
BF16 = mybir.dt.bfloat16

N_CORES = 8
P = 128              # SBUF partitions
FD_TOTAL = 32768     # free-dim elements per partition per core (4Mi total)
TILE_FD = 4096       # free-dim per tile
X_BUFS = 4           # input-tile ring depth (DMA prefetch runway)
OUT_BUFS = 3         # output-tile ring depth
TMP_BUFS = 2
STORE_SCALAR = True  # issue stores from the ACT HWDGE ring: the trigger sits
                     # right after Exp on the same engine so its wait is
                     # pre-satisfied, and stores never head-of-line-block
                     # input loads queued on the SP HWDGE ring
CAST_LOAD = False    # SWDGE (gpsimd-ring) input DMA with f32->bf16 cast.
                     # Measured NOT faster: the DMA bound tracks the f32
                     # bytes through the SDMA/HBM path regardless of the
                     # SBUF-side dtype, and the SWDGE cast path adds ~2-6us
                     # per pass, so plain HWDGE f32 loads win.
OUT_BF16 = True      # declare the DRAM output bf16 and upcast host-side:
                     # store bytes halve on BOTH sides of the DMA (unlike
                     # CAST_LOAD), cutting total DMA traffic 16+8=24 MiB and
                     # the DMA-bound floor by ~25%; Exp also reads/writes
                     # bf16 so the whole DVE chain runs in 2x/4x perf modes.
                     # Costs <=2^-9 output quantization, inside the budget.
N_ACT_SQ = 0 if OUT_BF16 else 3   # tiles (of 8) on the ACT-square path:
                     # balances ACT/DVE at ~70/70us for the f32-tail chain;
                     # the all-bf16 chain balances at ~59/62us with 0

# Quartic P4(u) ~= ln(i1e(e^u)) on u in [ln 0.1, ln 10.1], minimax-refit
# through the exact quantized evaluation chain for each pipeline config.
if OUT_BF16:
    ALPHA = 0.10368295013904572
    BETA = -0.012737303040921688
    C = -0.25116512179374695
    D = 0.22440478205680847
    BE = -1.572745680809021
elif CAST_LOAD:
    ALPHA = 0.1032966673374176
    BETA = -0.012588093057274818
    C = -0.2503528296947479
    D = 0.22434590756893158
    BE = -1.5741204023361206
else:
    ALPHA = 0.10338272154331207
    BETA = -0.012421127408742905
    C = -0.2503415644168854
    D = 0.2245168834924698
    BE = -1.5742369890213013

ACT_BIAS_CONSTS = [BETA, BE]

_CACHED_NC = None


def build_nc(reps: int = 1):
    nc = bass.Bass(trn_type="TRN2")
    x_ext = nc.declare_dram_parameter("x", [P, FD_TOTAL], F32, isOutput=False)
    o_ext = nc.declare_dram_parameter("o", [P, FD_TOTAL],
                                      BF16 if OUT_BF16 else F32, isOutput=True)

    # Register activation-bias constants as const APs, mirroring
    # Bass.__init__'s register_const_ap for 0.0/1.0.
    for i, v in enumerate(ACT_BIAS_CONSTS):
        tns = nc.alloc_sbuf_tensor(f"const-f32-bias{i}", [P, 1], F32)
        nc.gpsimd.memset(tns.ap(), v)
        nc.const_aps.aps[(F32, v)] = tns.ap()
    nc.all_engine_barrier()

    n_tiles = FD_TOTAL // TILE_FD
    store_engine = nc.scalar if STORE_SCALAR else nc.sync
    with tile.TileContext(nc) as tc:
        with (
            tc.tile_pool(name="iox", bufs=X_BUFS) as iox,
            tc.tile_pool(name="ioo", bufs=OUT_BUFS) as ioo,
            tc.tile_pool(name="tmp", bufs=TMP_BUFS) as tmp,
        ):
            for i in range(n_tiles * reps):
                i = i % n_tiles
                sl = bass.ts(i, TILE_FD)

                if CAST_LOAD:
                    x = iox.tile([P, TILE_FD], BF16, tag="x")
                    nc.gpsimd.dma_start(x[:], x_ext[:, sl])
                else:
                    x = iox.tile([P, TILE_FD], F32, tag="x")
                    nc.sync.dma_start(x[:], x_ext[:, sl])

                u = tmp.tile([P, TILE_FD], BF16, tag="u")
                nc.scalar.activation(u[:], x[:], AF.Ln)

                q = tmp.tile([P, TILE_FD], BF16, tag="q")
                if i % 8 >= 8 - N_ACT_SQ:
                    nc.scalar.activation(q[:], u[:], AF.Square,
                                         scale=ALPHA, bias=BETA)
                else:
                    t = tmp.tile([P, TILE_FD], BF16, tag="t")
                    nc.vector.tensor_scalar(t[:], u[:], ALPHA, BETA,
                                            ALU.mult, ALU.add)
                    nc.vector.tensor_tensor(q[:], t[:], t[:], ALU.mult)

                nc.vector.scalar_tensor_tensor(
                    q[:], q[:], C, u[:], ALU.add, ALU.mult)
                w_bf = CAST_LOAD or OUT_BF16
                w = tmp.tile([P, TILE_FD], BF16 if w_bf else F32, tag="w")
                nc.vector.scalar_tensor_tensor(
                    w[:], q[:], D, u[:], ALU.add, ALU.mult)

                out = ioo.tile([P, TILE_FD],
                               BF16 if OUT_BF16 else F32, tag="out")
                nc.scalar.activation(out[:], w[:], AF.Exp, bias=BE)

                store_engine.dma_start(o_ext[:, sl], out[:])

    _split_multi_waits(nc)
    return nc


# TPB compute-instruction ISA formats carry at most ONE sync-wait, but Tile's
# semaphore assignment can attach several (its wait minimality is per-proc,
# not transitive).  Hoist all but one wait onto an InstNoOp inserted right
# before the offending instruction on the same engine.
def _split_multi_waits(nc):
    for bb in nc.main_func.blocks:
        insts = bb.instructions
        i = 0
        while i < len(insts):
            inst = insts[i]
            si = inst.sync_info
            if si is not None and len(si.on_wait) > 1:
                for w in si.on_wait[:-1]:
                    nop = mybir.InstNoOp(
                        name=nc.get_next_instruction_name(),
                        text_hint="wait_split",
                        bass_nofuse=True,
                        engine=inst.engine,
                        sync_info=mybir.SyncInfo(on_wait=[w], on_update=[]),
                    )
                    insts.insert(i, nop)
                    i += 1
                si.on_wait = [si.on_wait[-1]]
            i += 1


def kernel(z: np.ndarray) -> np.ndarray:
    global _CACHED_NC
    assert z.shape == (32, 1024, 1024) and z.dtype == np.float32
    if _CACHED_NC is None:
        _CACHED_NC = build_nc()
    nc = _CACHED_NC

    per_core = 32 // N_CORES
    shards = z.reshape(N_CORES, per_core * 1024 * 1024).reshape(N_CORES, P, FD_TOTAL)
    in_maps = [{"x": np.ascontiguousarray(shards[k])} for k in range(N_CORES)]
    res = run_bass_kernel_spmd(nc, in_maps, list(range(N_CORES))).results
    out = np.concatenate(
        [res[k]["o"].astype(np.float32).reshape(per_core, 1024, 1024)
         for k in range(N_CORES)], axis=0
    )
    return out.astype(np.float32)
